# revision 5
# baseline (speedup 1.0000x reference)
# InternLM2-7B decode-step paged attention on 8 Trainium2 NeuronCores.
#
# Sharding (tensor-parallel, per the source hooks):
#   - wqkv column-sharded: core c gets q heads 4c..4c+3 and kv head c
#   - wo row-sharded: core c gets rows for q heads 4c..4c+3
#   - KV cache sharded along the kv-head dim: core c gets head c
#   - output projection partials summed on the host (the all-reduce)
#
# Host-side prep (pure data movement / tiny math):
#   - paged-cache gather via block_offsets (a permutation of blocks),
#     head-slice, cast to bf16, and (for K) transpose to [B, HD, L]
#   - RoPE cos/sin tables from position_ids_1d
#   - additive validity mask from kv_seqlens
#
# Device math per core (see _emit):
#   qkv = hT.T @ wqkv_shard; rope(q, k); scores = (q*scale)_bf16 @ kT_bf16
#   with the new token's score computed separately in f32; masked batched
#   softmax over [128 rows = 32 seqs x 4 heads, 4097]; out = probs @ v + p_new*v_new;
#   y_partial = out @ wo_shard (f32).
import os
import sys

for _p in (
    "/opt/trn_rl_repo",
    "/root/.axon_site",
    "/root/.axon_site/_ro/trn_rl_repo",
    "/root/.axon_site/_ro/pypackages",
):
    if os.path.isdir(_p) and _p not in sys.path:
        sys.path.append(_p)

import numpy as np
import ml_dtypes

BF16NP = ml_dtypes.bfloat16

import concourse.bass as bass
from concourse import bacc
import concourse.mybir as mybir
import concourse.tile as tile
from concourse.masks import make_identity

B = 32          # batch (decoding sequences)
H = 32          # query heads
KVH = 8         # kv heads
G = 4           # query heads per kv head (= per core)
HD = 128        # head dim
D = 4096        # model dim
W = (G + 2) * HD  # per-core qkv shard width = 768
L = 4096        # kv positions per sequence
BLOCK = 64
NBLK = 64
NCORES = 8
THETA = 1e6

F32 = mybir.dt.float32
BF16 = mybir.dt.bfloat16
SCALE = 1.0 / float(np.sqrt(HD))
NEG = -1.0e30


def _emit(nc, tc, hT, wq, wo, kT, vv, cs, mk, y, B_, L_):
    """Emit the per-core program. All SBUF compute APs start at partition 0;
    partition scatter/gather is done with SBUF<->SBUF DMAs (exempt from the
    32-strip start-partition rule)."""
    import contextlib

    R = G * B_            # score rows (seq-major: row = s*G + h)
    NT = L_ // 512        # kT 512-col chunks
    NA = L_ // 128        # 128-pos tiles
    KT_ = D // 128        # contraction tiles for the qkv projection
    X = mybir.AxisListType.X

    with contextlib.ExitStack() as ctx:
        singles = ctx.enter_context(tc.tile_pool(name="singles", bufs=1))
        wqp = ctx.enter_context(tc.tile_pool(name="wqp", bufs=3))
        ktp = ctx.enter_context(tc.tile_pool(name="ktp", bufs=4))
        vtp = ctx.enter_context(tc.tile_pool(name="vtp", bufs=4))
        stg = ctx.enter_context(tc.tile_pool(name="stg", bufs=3))
        psp = ctx.enter_context(tc.tile_pool(name="psp", bufs=2, space="PSUM"))
        psp1 = ctx.enter_context(tc.tile_pool(name="psp1", bufs=1, space="PSUM"))

        ident = singles.tile([128, 128], F32)
        make_identity(nc, ident)
        ident_bf = singles.tile([128, 128], BF16)
        make_identity(nc, ident_bf)

        hT_sb = singles.tile([128, KT_, B_], F32)
        nc.sync.dma_start(hT_sb, hT.rearrange("(t p) b -> p t b", p=128))
        cs_sb = singles.tile([B_, HD], F32)
        nc.sync.dma_start(cs_sb, cs)
        mask_sb = singles.tile([R, L_ + 1], F32)
        nc.sync.dma_start(mask_sb, mk)
        wo_sb = singles.tile([128, G, D], F32)
        nc.sync.dma_start(wo_sb, wo.rearrange("(h p) n -> p h n", p=128))

        # ---- fused QKV projection: qkv[B_, W] = hT.T @ wq ----
        ps_q0 = psp.tile([128, 384], F32, tag="qy")
        ps_q1 = psp.tile([128, 384], F32, tag="qy")
        for t in range(KT_):
            wt = wqp.tile([128, W], F32, tag="wt")
            nc.sync.dma_start(wt, wq[t * 128 : (t + 1) * 128, :])
            nc.tensor.matmul(ps_q0[:B_, :], lhsT=hT_sb[:, t, :], rhs=wt[:, 0:384],
                             start=(t == 0), stop=(t == KT_ - 1))
            nc.tensor.matmul(ps_q1[:B_, :], lhsT=hT_sb[:, t, :], rhs=wt[:, 384:W],
                             start=(t == 0), stop=(t == KT_ - 1))
        qkv_sb = singles.tile([B_, W], F32)
        nc.vector.tensor_copy(qkv_sb[:, 0:384], ps_q0[:B_, :])
        nc.vector.tensor_copy(qkv_sb[:, 384:W], ps_q1[:B_, :])

        # ---- RoPE on q (G heads) and k (1 head); v passthrough ----
        q_sb = singles.tile([B_, G * HD], F32)
        k_sb = singles.tile([B_, HD], F32)
        v_sb = singles.tile([B_, HD], F32)
        nc.vector.tensor_copy(v_sb, qkv_sb[:, (G + 1) * HD : (G + 2) * HD])
        cosv = cs_sb[:, 0:64]
        sinv = cs_sb[:, 64:128]
        for j in range(G + 1):
            src = qkv_sb[:, j * HD : (j + 1) * HD]
            dst = q_sb[:, j * HD : (j + 1) * HD] if j < G else k_sb[:, :]
            a = src[:, 0:64]
            b = src[:, 64:128]
            t1 = stg.tile([B_, 64], F32, tag="rt1")
            t2 = stg.tile([B_, 64], F32, tag="rt2")
            nc.vector.tensor_mul(t1, a, cosv)
            nc.vector.tensor_mul(t2, b, sinv)
            nc.vector.tensor_sub(dst[:, 0:64], t1, t2)
            t3 = stg.tile([B_, 64], F32, tag="rt1")
            t4 = stg.tile([B_, 64], F32, tag="rt2")
            nc.vector.tensor_mul(t3, b, cosv)
            nc.vector.tensor_mul(t4, a, sinv)
            nc.vector.tensor_add(dst[:, 64:128], t3, t4)

        # ---- qT (pre-scaled, bf16): qT_buf[d, s, h] ----
        qT_buf = singles.tile([128, B_, G], BF16)
        for h in range(G):
            ps_t = psp1.tile([128, 128], F32, tag="tr")
            nc.tensor.transpose(ps_t[:, :B_], q_sb[:, h * HD : (h + 1) * HD],
                                ident[:B_, :B_])
            nc.vector.tensor_scalar_mul(out=qT_buf[:, :, h], in0=ps_t[:, :B_],
                                        scalar1=SCALE)

        # ---- new-token score (f32): row-major copies via DMA reshape ----
        q_row = singles.tile([R, HD], F32)
        nc.gpsimd.dma_start(q_row, q_sb[:, :])
        k_rep = singles.tile([R, HD], F32)
        nc.gpsimd.dma_start(k_rep, k_sb[:, None, :].to_broadcast((B_, G, HD)))
        v_rep = singles.tile([R, HD], F32)
        nc.gpsimd.dma_start(v_rep, v_sb[:, None, :].to_broadcast((B_, G, HD)))

        scores = singles.tile([R, L_ + 1], F32)
        tsn = singles.tile([R, HD], F32)
        nc.vector.tensor_mul(tsn, q_row, k_rep)
        nc.vector.reduce_sum(out=scores[:, L_ : L_ + 1], in_=tsn, axis=X)
        nc.scalar.mul(scores[:, L_ : L_ + 1], scores[:, L_ : L_ + 1], SCALE)

        # ---- cache scores: per (seq, 512-chunk) matmul, staged to rows ----
        for s in range(B_):
            for tg in range(NT):
                kt_t = ktp.tile([128, 512], BF16, tag="kt")
                nc.sync.dma_start(kt_t, kT[s, :, tg * 512 : (tg + 1) * 512])
                ps_sc = psp.tile([128, 512], F32, tag="sc")
                nc.tensor.matmul(ps_sc[:G, :], lhsT=qT_buf[:, s, :], rhs=kt_t,
                                 start=True, stop=True)
                sct = stg.tile([G, 512], F32, tag="sct")
                nc.vector.tensor_copy(sct, ps_sc[:G, :])
                nc.sync.dma_start(scores[s * G : (s + 1) * G,
                                         tg * 512 : (tg + 1) * 512], sct)

        nc.vector.tensor_add(scores, scores, mask_sb)

        # ---- batched softmax over [R, L_+1] ----
        mx = singles.tile([R, 1], F32)
        ngm = singles.tile([R, 1], F32)
        sm = singles.tile([R, 1], F32)
        rc = singles.tile([R, 1], F32)
        pnew = singles.tile([R, 1], F32)
        probs = singles.tile([R, L_ + 1], BF16)
        nc.vector.reduce_max(out=mx, in_=scores, axis=X)
        nc.scalar.mul(ngm, mx, -1.0)
        nc.scalar.activation(out=probs, in_=scores,
                             func=mybir.ActivationFunctionType.Exp,
                             bias=ngm, scale=1.0, accum_out=sm)
        nc.scalar.activation(out=pnew, in_=scores[:, L_ : L_ + 1],
                             func=mybir.ActivationFunctionType.Exp,
                             bias=ngm, scale=1.0)
        nc.vector.reciprocal(rc, sm)

        # ---- transpose probs into attnT[pos, row] tiles ----
        attnT = singles.tile([128, NA, R], BF16)
        for t in range(NA):
            ps_t = psp.tile([128, 128], BF16, tag="trb")
            nc.tensor.transpose(ps_t[:, :R], probs[:, t * 128 : (t + 1) * 128],
                                ident_bf[:R, :R])
            nc.vector.tensor_copy(attnT[:, t, :], ps_t[:, :R])

        # ---- V accumulation: out[s*G+h, d] ----
        out_all = singles.tile([R, HD], F32)
        for s in range(B_):
            ps_o = psp1.tile([128, HD], F32, tag="po")
            vs = vv[s, :, :].rearrange("(a p) d -> p a d", p=128)
            for tg in range(NA // 4):
                vt = vtp.tile([128, 4, HD], BF16, tag="vt")
                nc.sync.dma_start(vt, vs[:, tg * 4 : (tg + 1) * 4, :])
                for j in range(4):
                    ti = tg * 4 + j
                    nc.tensor.matmul(ps_o[:G, :],
                                     lhsT=attnT[:, ti, s * G : (s + 1) * G],
                                     rhs=vt[:, j, :],
                                     start=(ti == 0), stop=(ti == NA - 1))
            ost = stg.tile([G, HD], F32, tag="ost")
            nc.vector.tensor_copy(ost, ps_o[:G, :])
            nc.sync.dma_start(out_all[s * G : (s + 1) * G, :], ost)

        # normalize + new-token contribution
        nc.vector.tensor_scalar_mul(out=out_all, in0=out_all, scalar1=rc)
        pn2 = singles.tile([R, 1], F32)
        tvn = singles.tile([R, HD], F32)
        nc.vector.tensor_mul(pn2, pnew, rc)
        nc.vector.tensor_scalar_mul(out=tvn, in0=v_rep, scalar1=pn2)
        nc.vector.tensor_add(out_all, out_all, tvn)

        # ---- transpose out_all -> outT[d, h, s] (f32) ----
        ps_ot = psp1.tile([128, 128], F32, tag="tr")
        nc.tensor.transpose(ps_ot[:, :R], out_all, ident[:R, :R])
        outT = singles.tile([128, G, B_], F32)
        nc.vector.tensor_copy(outT.rearrange("p h s -> p s h"),
                              ps_ot[:, :R].rearrange("p (s h) -> p s h", h=G))

        # ---- output projection partial: y = outT.T @ wo_shard ----
        y_sb = singles.tile([B_, D], F32)
        for n in range(D // 512):
            ps_y = psp.tile([128, 512], F32, tag="qy")
            for h in range(G):
                nc.tensor.matmul(ps_y[:B_, :], lhsT=outT[:, h, :],
                                 rhs=wo_sb[:, h, n * 512 : (n + 1) * 512],
                                 start=(h == 0), stop=(h == G - 1))
            nc.vector.tensor_copy(y_sb[:, n * 512 : (n + 1) * 512], ps_y[:B_, :])
        nc.sync.dma_start(y, y_sb)


_NC_CACHE = None


def build_bass():
    global _NC_CACHE
    if _NC_CACHE is not None:
        return _NC_CACHE
    nc = bacc.Bacc("TRN2")
    hT = nc.dram_tensor("hT", [D, B], F32, kind="ExternalInput")
    wq = nc.dram_tensor("wq", [D, W], F32, kind="ExternalInput")
    wo = nc.dram_tensor("wo", [G * HD, D], F32, kind="ExternalInput")
    kT = nc.dram_tensor("kT", [B, HD, L], BF16, kind="ExternalInput")
    vv = nc.dram_tensor("vv", [B, L, HD], BF16, kind="ExternalInput")
    cs = nc.dram_tensor("cs", [B, HD], F32, kind="ExternalInput")
    mk = nc.dram_tensor("mk", [G * B, L + 1], F32, kind="ExternalInput")
    y = nc.dram_tensor("y", [B, D], F32, kind="ExternalOutput")
    with tile.TileContext(nc) as tc:
        _emit(nc, tc, hT[:, :], wq[:, :], wo[:, :], kT[:, :, :], vv[:, :, :],
              cs[:, :], mk[:, :], y[:, :], B, L)
    nc.finalize()  # runs Bacc.compile(): wait legalization, reg alloc, DCE
    _NC_CACHE = nc
    return nc


def make_host_inputs(hidden_states, wqkv, wo, k_cache, v_cache,
                     position_ids_1d, block_offsets, kv_seqlens):
    """Shard + preprocess full inputs into 8 per-core in_maps."""
    hidden_states = np.asarray(hidden_states, dtype=np.float32)
    wqkv = np.asarray(wqkv, dtype=np.float32)
    wo = np.asarray(wo, dtype=np.float32)
    k_cache = np.asarray(k_cache, dtype=np.float32)
    v_cache = np.asarray(v_cache, dtype=np.float32)
    position_ids_1d = np.asarray(position_ids_1d, dtype=np.int32)
    block_offsets = np.asarray(block_offsets, dtype=np.int32)
    kv_seqlens = np.asarray(kv_seqlens, dtype=np.int32)

    hT = np.ascontiguousarray(hidden_states.T)  # [D, B]

    # RoPE tables (f32, matching the reference convention)
    inv_freq = (1.0 / (THETA ** (np.arange(0, HD, 2, dtype=np.float64) / HD)))
    ang = position_ids_1d.astype(np.float64)[:, None] * inv_freq[None, :]
    cs_host = np.concatenate(
        [np.cos(ang), np.sin(ang)], axis=1).astype(np.float32)  # [B, 128]

    # additive mask over [rows = s*G+h, L+1]; cache col j valid iff
    # j < seqlen-1 (the cache row at seqlen-1 is replaced by the new token,
    # which lives in the extra column L and is always valid)
    j = np.arange(L, dtype=np.int64)[None, :]
    valid = j < (kv_seqlens.astype(np.int64)[:, None] - 1)
    mask_seq = np.where(valid, 0.0, NEG).astype(np.float32)  # [B, L]
    mask_seq = np.concatenate(
        [mask_seq, np.zeros((B, 1), np.float32)], axis=1)  # [B, L+1]
    mask = np.repeat(mask_seq, G, axis=0)  # [G*B, L+1]

    # paged gather: per-sequence kv via block table (a permutation of blocks)
    ident_blocks = np.array_equal(block_offsets.ravel(),
                                  np.arange(B * NBLK, dtype=np.int64))

    kx = np.moveaxis(k_cache, 2, 0)  # [KVH, NUM_BLOCKS, BLOCK, HD] (view)
    vx = np.moveaxis(v_cache, 2, 0)

    in_maps = []
    for c in range(NCORES):
        if ident_blocks:
            kg = kx[c].reshape(B, L, HD)
            vg = vx[c].reshape(B, L, HD)
        else:
            kg = kx[c][block_offsets].reshape(B, L, HD)
            vg = vx[c][block_offsets].reshape(B, L, HD)
        kT_c = np.ascontiguousarray(
            kg.astype(BF16NP).transpose(0, 2, 1))          # [B, HD, L]
        v_c = np.ascontiguousarray(vg.astype(BF16NP))      # [B, L, HD]
        wq_c = np.ascontiguousarray(np.concatenate([
            wqkv[:, c * G * HD : (c + 1) * G * HD],
            wqkv[:, H * HD + c * HD : H * HD + (c + 1) * HD],
            wqkv[:, (H + KVH) * HD + c * HD : (H + KVH) * HD + (c + 1) * HD],
        ], axis=1))                                        # [D, W]
        wo_c = np.ascontiguousarray(
            wo[c * G * HD : (c + 1) * G * HD, :])          # [G*HD, D]
        in_maps.append(dict(hT=hT, wq=wq_c, wo=wo_c, kT=kT_c, vv=v_c,
                            cs=cs_host, mk=mask))
    return in_maps


def kernel(**inputs):
    from concourse.bass_utils import run_bass_kernel_spmd

    in_maps = make_host_inputs(
        inputs["hidden_states"], inputs["wqkv"], inputs["wo"],
        inputs["k_cache"], inputs["v_cache"], inputs["position_ids_1d"],
        inputs["block_offsets"], inputs["kv_seqlens"])
    nc = build_bass()
    res = run_bass_kernel_spmd(nc, in_maps, core_ids=list(range(NCORES)))
    y = np.zeros((B, D), dtype=np.float32)
    for r in res.results:
        y += np.asarray(r["y"], dtype=np.float32)
    return y
